# revision 8
# baseline (speedup 1.0000x reference)
"""Trainium2 Bass kernel for nn_DeepSignatureModel (depth-2 signature model).

Self-contained: hardcodes shapes from the problem spec.
  x: (64, 1024, 5) f32, lengths: (64,) int64  ->  out: (64, 32) f32

v2 redesign (vs the 255us baseline):
  - g expansion runs as 8-tile tile_position passes (4 row groups x 2 col
    groups, K=22 M=64), ~4x concurrent: 4 PE streams/batch instead of 16.
  - 64-row g chunks; the vh (level-1) contraction is folded into chunk 7
    via an ones-row in mrep, removing 4 u streams/batch.
  - u projections are col-tiled (0,0)/(0,64) concurrent pairs with the
    +1/+2/+3 time shifts folded into the rhs access pattern; u2+u3
    accumulate into one PSUM region, so the DVE merge is 3 aligned adds.
  - w2b bias removed from the device (K=64); reinjected via a -b2
    signature basepoint prefix and a host-side F2 += outer(b2, F1) term.
  - scan chained A->B via AP initial; engine rebalance (GP scans m'/dx/y,
    ACT/DVE split stages+muls).
  - f32r convs stay in the baseline base-0 [64, T] layout (fp32-mode
    matmuls cannot be tile-packed).
"""

import numpy as np

import concourse.bass as bass
import concourse.bacc as bacc
import concourse.mybir as mybir
import concourse.tile as tile
from concourse.bass_utils import run_bass_kernel_spmd

# ---- problem constants ----
K = 4
B, S, CIN = 64, 1024, 5
H1, H2 = 64, 16
OUT = 32
C1 = 22                    # channels entering signature1
L1 = S - K + 1             # 1021
L2 = L1 - K + 1            # 1018
NB = 8                     # batches per core
NCORES = 8
T = 1024
NCH = 8                    # 64-row g chunks (506 = 484 + 22 dx rows, padded)
GW = 1032                  # g tile cols (1024 + shift pad)
L2B = L2 - 512             # 506 valid scan/merge cols in half B
# device h channel order: conv-out(16), x(5), time(1); PERM[new_row] = orig_chan
PERM = list(range(6, 22)) + list(range(0, 5)) + [5]
F32 = mybir.dt.float32
F32R = mybir.dt.float32r
BF16 = mybir.dt.bfloat16

_COMPILED = None
TRACE = False
LAST = None

# packed-constant layouts: name -> (row_off, rows, col_off, cols)
CBF_LAYOUT = {
    "ri0": (0, 128, 0, 128),
    "ri1": (0, 128, 128, 128),
    "rj0": (0, 128, 256, 128),
    "rj1": (0, 128, 384, 128),
    "va0": (0, 128, 512, 128),
    "va1": (0, 128, 640, 128),
    "va2": (0, 128, 768, 128),
    "va3": (0, 128, 896, 128),
    "vd0": (0, 128, 1024, 128),
    "vd1": (0, 128, 1152, 128),
    "vd2": (0, 128, 1280, 128),
    "vd3": (0, 128, 1408, 128),
    "w2b2": (0, 64, 1536, 16),
    "onec": (0, 128, 1552, 1),
    "cw0": (0, 52, 1553, 64),
    "cw1": (0, 128, 1617, 64),
    "cw2": (0, 128, 1681, 16),
}
CBF_COLS = 1697
CFR_LAYOUT = {
    "w0p": (0, 20, 0, 64),
    "w1a": (0, 64, 64, 64),
    "w2a": (0, 64, 128, 16),
    "w1b": (0, 64, 144, 64),
    "ssh": (0, 128, 208, 128),
    "s127": (0, 128, 336, 128),
    "b2negr": (0, 1, 464, 128),
}
CFR_COLS = 592
# cf32: rows 0/1 = time row halves, row 3 = 0.5 row; cols 512+ = bias columns
CF32_LAYOUT = {
    "trowA": (0, 1, 0, 512),
    "trowB": (1, 1, 0, 512),
    "halfrow": (3, 1, 0, 512),
    "b0c": (0, 64, 512, 1),
    "b1c": (0, 64, 513, 1),
    "b1bc": (0, 64, 514, 1),
    "ba0c": (0, 64, 515, 1),
    "b2c": (0, 48, 516, 1),
    "zc23": (0, 23, 517, 1),
    "b0r2": (0, 128, 518, 1),
    "b1r2": (0, 128, 519, 1),
}
CF32_COLS = 520


def build_program():
    nc = bacc.Bacc()

    def inp(name, shape, dt_=F32):
        return nc.declare_dram_parameter(name, list(shape), dt_, isOutput=False)

    xs_d = inp("xs", (NB, 52, 512), BF16)        # im2col halves-on-rows
    xh_d = inp("xh", (NB, 5, T), F32)            # x rows for h (k=3)
    mask_d = inp("masktm", (128, NB * 8), F32R)  # j-major: col = 8*j + b
    cbf_d = inp("cbf", (128, CBF_COLS), BF16)
    cfr_d = inp("cfr", (128, CFR_COLS), F32R)
    cf32_d = inp("cf32", (128, CF32_COLS), F32)
    f2o_d = nc.declare_dram_parameter("f2o", [128, 128], F32, isOutput=True)
    f1o_d = nc.declare_dram_parameter("f1o", [1, T], F32, isOutput=True)

    Relu = mybir.ActivationFunctionType.Relu
    Copy = mybir.ActivationFunctionType.Copy
    Ident = mybir.ActivationFunctionType.Identity
    ADD = mybir.AluOpType.add
    SUB = mybir.AluOpType.subtract
    MUL = mybir.AluOpType.mult
    MAX = mybir.AluOpType.max

    with tile.TileContext(nc) as tc:
        with (
            tc.tile_pool(name="const", bufs=1) as cpool,
            tc.tile_pool(name="xin", bufs=3) as xpool,
            tc.tile_pool(name="mst", bufs=12) as mpool,
            tc.tile_pool(name="alls", bufs=1) as spool,
            tc.tile_pool(name="slots", bufs=1) as slpool,
            tc.tile_pool(name="work", bufs=6, space="PSUM") as wk_ps,
            tc.tile_pool(name="uap", bufs=1, space="PSUM") as uA_ps,
        ):
            # ---- load packed constants ----
            cfr = cpool.tile([128, CFR_COLS], F32R, tag="cfr")
            nc.sync.dma_start(out=cfr[:], in_=cfr_d.ap())
            cf32 = cpool.tile([128, CF32_COLS], F32, tag="cf32")
            nc.sync.dma_start(out=cf32[:], in_=cf32_d.ap())
            xsb0 = xpool.tile([52, 512], BF16, tag="xsb")
            nc.sync.dma_start(out=xsb0[:], in_=xs_d.ap()[0])
            cbf = cpool.tile([128, CBF_COLS], BF16, tag="cbf")
            nc.sync.dma_start(out=cbf[:], in_=cbf_d.ap())
            maskt = cpool.tile([128, NB * 8], F32R, tag="maskt")
            nc.sync.dma_start(out=maskt[:], in_=mask_d.ap())


            def vbf(name):
                r0, r, o, c = CBF_LAYOUT[name]
                return cbf[r0 : r0 + r, o : o + c]

            def vfr(name):
                r0, r, o, c = CFR_LAYOUT[name]
                return cfr[r0 : r0 + r, o : o + c]

            ri = [vbf("ri0"), vbf("ri1")]
            rj = [vbf("rj0"), vbf("rj1")]
            va = [vbf(f"va{q}") for q in range(4)]
            vd = [vbf(f"vd{q}") for q in range(4)]
            w2b2, onec = vbf("w2b2"), vbf("onec")
            cw0, cw1, cw2 = vbf("cw0"), vbf("cw1"), vbf("cw2")
            w0p, w1a, w2a, w1b = (vfr("w0p"), vfr("w1a"), vfr("w2a"),
                                  vfr("w1b"))
            ssh, s127 = vfr("ssh"), vfr("s127")
            b0c = cf32[0:64, 512:513]
            b1c = cf32[0:64, 513:514]
            b1bc = cf32[0:64, 514:515]
            ba0c = cf32[0:64, 515:516]
            b2c = cf32[0:48, 516:517]
            b0r2 = cf32[0:128, 518:519]
            b1r2 = cf32[0:128, 519:520]
            zc23 = cf32[0:23, 517:518]

            zeros = cpool.tile([128, 512], F32, tag="zeros")
            nc.vector.memset(zeros[:], 0.0)
            zeros1k = cpool.tile([64, 1024], F32, tag="zeros1k")
            nc.vector.memset(zeros1k[:], 0.0)

            # all-batch tiles for signature2 (j-major time blocks)
            h2all = spool.tile([128, 128 + NB * 128], F32R, tag="h2all")
            d2f = spool.tile([128, NB * 128], F32, tag="d2f")
            d2 = spool.tile([128, NB * 128], BF16, tag="d2")
            m2p = spool.tile([128, NB * 128], BF16, tag="m2p")
            f2sb = spool.tile([128, 128], F32, tag="f2sb")
            f1sb = spool.tile([1, T], F32, tag="f1sb")
            nc.vector.memset(h2all[:, 0:128].bitcast(F32), 0.0)
            # basepoint prefix row = -b2 (bias reinjection; host adds
            # outer(b2, F1) back)
            nc.sync.dma_start(
                out=h2all[127:128, 0:128], in_=cfr_d.ap()[0:1, 464:592]
            )

            def mm(out, lhsT, rhs, tp=None, start=True, stop=True):
                nc.tensor.matmul(out, lhsT, rhs, start=start, stop=stop,
                                 tile_position=tp, skip_group_check=True)

            # ---- persistent slots ----
            def mkslots(shape, dt_, tag, n=2):
                return [slpool.tile(list(shape), dt_, tag=f"{tag}{i}",
                                    name=f"{tag}{i}") for i in range(n)]

            r0s = mkslots((128, 512), BF16, "r0")
            r1s = mkslots((128, 512), BF16, "r1")
            hs = mkslots((23, T), F32, "h")
            mfulls = mkslots((23, T), BF16, "mfull")
            dxfulls = mkslots((23, T), BF16, "dxfull")
            mreps = mkslots((128, 512), BF16, "mrep")
            dxreps = mkslots((128, 512), BF16, "dxrep")
            gs = [[slpool.tile([128, GW], BF16, tag=f"g{i}_{q}",
                               name=f"g{i}_{q}") for q in range(4)]
                  for i in range(2)]
            ypres2 = mkslots((64, 1024), F32, "ypre2")
            scns = mkslots((64, 1024), F32, "scn")
            t1s = mkslots((64, 1024), F32, "t1")
            ypres = mkslots((64, 1024), F32, "ypre")
            ys = mkslots((64, T), F32R, "y", n=3)
            r2s = mkslots((64, T), BF16, "r2")

            # one-time presets
            for hsl in hs:
                nc.sync.dma_start(out=hsl[21:22, 0:512],
                                  in_=cf32_d.ap()[0:1, 0:512])
                nc.sync.dma_start(out=hsl[21:22, 512:1024],
                                  in_=cf32_d.ap()[1:2, 0:512])
                nc.sync.dma_start(out=hsl[22:23, 0:512],
                                  in_=cf32_d.ap()[3:4, 0:512])
                nc.sync.dma_start(out=hsl[22:23, 512:1024],
                                  in_=cf32_d.ap()[3:4, 0:512])
                nc.vector.memset(hsl[0:16, L1:T], 0.0)
            for t_ in mfulls + dxfulls:
                nc.gpsimd.memset(t_[:, L1:T], 0.0)
            for t_ in ys:
                nc.vector.memset(t_[:, L2:T].bitcast(F32), 0.0)
            for t_ in r2s:
                nc.gpsimd.memset(t_[:, L2:T], 0.0)
            for gsl in gs:
                for g in gsl:
                    nc.gpsimd.memset(g[:, 1024:GW], 0.0)

            # ================= pipeline stages =================
            def emit_convs(b):
                """augment1 conv stack (channel-major, base-0 f32r)"""
                if b == 0:
                    xsb = xsb0
                else:
                    xsb = xpool.tile([20, T], F32R, tag="xsb")
                    nc.sync.dma_start(out=xsb[:], in_=xs_d.ap()[b])

                r0 = r0s[b % 2]
                r1 = r1s[b % 2]
                h = hs[b % 2]
                for h0 in (0, 512):
                    ps0 = wk_ps.tile([64, 512], F32, tag="ex", name="ps0")
                    mm(ps0[:], w0p[:], xsb[:, h0 : h0 + 512])
                    n = min(512, L1 - h0)
                    nc.scalar.activation(r0[:, h0 : h0 + n], ps0[:, 0:n],
                                         Relu, bias=b0c)
                for h0 in (0, 512):
                    ps1 = wk_ps.tile([64, 512], F32, tag="ex", name="ps1")
                    mm(ps1[:], w1a[:], r0[:, h0 : h0 + 512])
                    n = min(512, L1 - h0)
                    nc.scalar.activation(r1[:, h0 : h0 + n], ps1[:, 0:n],
                                           Relu, bias=b1c)
                for h0 in (0, 512):
                    psh = wk_ps.tile([16, 512], F32, tag="ex", name="psh")
                    mm(psh[:], w2a[:], r1[:, h0 : h0 + 512])
                    n = min(512, L1 - h0)
                    nc.scalar.activation(h[0:16, h0 : h0 + n], psh[:, 0:n],
                                         Ident, bias=b2c)
                nc.sync.dma_start(out=h[16:21, :],
                                  in_=xsb[15:20, :].bitcast(F32))

            self_state = {}
            self_mstg = {}

            def emit_mdx(b):
                """m'/dx prep + row-group scatter (one iteration early)"""
                h = hs[b % 2]
                mfull = mfulls[b % 2]
                dxfull = dxfulls[b % 2]
                mrep = mreps[b % 2]
                dxrep = dxreps[b % 2]
                gp = nc.gpsimd

                gp.tensor_tensor(mfull[:, 1:L1], h[:, 1:L1], h[:, 0 : L1 - 1],
                                 ADD)
                gp.tensor_tensor(dxfull[:, 1:L1], h[:, 1:L1],
                                 h[:, 0 : L1 - 1], SUB)
                gp.tensor_tensor(mfull[:, 0:1], h[:, 0:1], zc23, ADD)
                gp.tensor_tensor(dxfull[:, 0:1], h[:, 0:1], zc23, SUB)
                for rb in range(4):
                    c0 = 512 * (rb % 2)
                    nc.gpsimd.dma_start(
                        out=mrep[32 * rb : 32 * rb + 23, :],
                        in_=mfull[:, c0 : c0 + 512])
                    eng = nc.gpsimd if rb % 2 == 0 else nc.sync
                    eng.dma_start(
                        out=dxrep[32 * rb : 32 * rb + 23, :],
                        in_=dxfull[:, c0 : c0 + 512])

            def emit_mid1(b):
                h = hs[b % 2]
                mrep = mreps[b % 2]
                dxrep = dxreps[b % 2]
                gq = gs[b % 2]
                self_state[b] = (h, mrep, dxrep, gq)
                emit_exp_m(b, 0)

            def emit_exp_m(b, p):
                h, mrep, dxrep, gq = self_state[b]
                mbk = [wk_ps.tile([128, 512], F32, tag="ex",
                                  name=f"mbk{p}_{i}") for i in range(4)]
                for rb in range(4):
                    for cg in range(2):
                        mm(mbk[rb][64 * cg : 64 * cg + 64, :],
                           ri[p][32 * rb : 32 * rb + 23,
                                 64 * cg : 64 * cg + 64],
                           mrep[32 * rb : 32 * rb + 23, :],
                           (32 * rb, 64 * cg))
                mstg = [mpool.tile([128, 512], BF16, tag="mstg",
                                   name=f"mstg{p}_{i}") for i in range(4)]
                for i in range(4):
                    if i == 3:
                        nc.vector.tensor_copy(mstg[i][:], mbk[i][:])
                    else:
                        nc.scalar.activation(mstg[i][:], mbk[i][:], Copy)
                self_mstg[(b, p)] = mstg

            def emit_exp_d(b, p):
                h, mrep, dxrep, gq = self_state[b]
                mstg = self_mstg.pop((b, p))
                dbk = [wk_ps.tile([128, 512], F32, tag="ex",
                                  name=f"dbk{p}_{i}") for i in range(4)]
                for rb in range(4):
                    for cg in range(2):
                        mm(dbk[rb][64 * cg : 64 * cg + 64, :],
                           rj[p][32 * rb : 32 * rb + 23,
                                 64 * cg : 64 * cg + 64],
                           dxrep[32 * rb : 32 * rb + 23, :],
                           (32 * rb, 64 * cg))
                for rb in range(4):
                    g = gq[2 * p + rb // 2]
                    c0 = 512 * (rb % 2)
                    nc.vector.tensor_tensor(
                        g[:, c0 : c0 + 512], mstg[rb][:], dbk[rb][:], MUL)

            def emit_mid2(b):
                h, mrep, dxrep, gq = self_state.pop(b)
                gp = nc.gpsimd
                # u projections: col-tiled pairs, shifts folded into rhs
                psA = uA_ps.tile([128, 1024], F32, tag="uA")
                psD = wk_ps.tile([128, 512], F32, tag="ex", name="psD")
                for c0 in (0, 512):
                    for q in range(4):
                        st, sp = q == 0, q == 3
                        mm(psA[0:64, c0 : c0 + 512], va[q][:, 0:64],
                           gq[q][:, c0 : c0 + 512], (0, 0), st, sp)
                        mm(psA[64:128, c0 : c0 + 512], va[q][:, 64:128],
                           gq[q][:, c0 + 1 : c0 + 513], (0, 64), st, sp)
                # u2+u3 accumulate together: halfA -> psD[0:64], halfB -> [64:]
                for q in range(4):
                    for d in (2, 3):
                        st = q == 0 and d == 2
                        sp = q == 3 and d == 3
                        w = vd[q][:, 0:64] if d == 2 else vd[q][:, 64:128]
                        mm(psD[0:64, :], w, gq[q][:, d : d + 512], (0, 0),
                           st, sp)
                        mm(psD[64:128, :], w, gq[q][:, 512 + d : 1024 + d],
                           (0, 64), st, sp)

                # scan on DVE (u0 is contiguous in psA)
                scn = scns[b % 2]
                nc.vector.tensor_tensor_scan(scn[:, 0:L2], psA[0:64, 0:L2],
                                             zeros1k[0:64, 0:L2], 0.0,
                                             ADD, ADD)
                # merge: ypre = scn + u1 + (u2+u3)
                t1 = t1s[b % 2]
                ypre = ypres[b % 2]
                nc.vector.tensor_tensor(
                    t1[:, 0:L2], scn[:, 0:L2], psA[64:128, 0:L2], ADD)
                nc.vector.tensor_tensor(
                    ypre[:, 0:512], t1[:, 0:512], psD[0:64, :], ADD)
                nc.vector.tensor_tensor(
                    ypre[:, 512:L2], t1[:, 512:L2], psD[64:128, 0:L2B], ADD)
                y = ys[b % 3]
                nc.vector.tensor_scalar(y[:, 0:L2], ypre[:, 0:L2], ba0c,
                                        0.0, ADD, MAX)

            def emit_late(b):
                """augment2 pointwise convs + j-major h2 block"""
                y = ys[b % 3]
                r2 = r2s[b % 2]
                for h0 in (0, 512):
                    psY = wk_ps.tile([64, 512], F32, tag="ex", name="psY")
                    mm(psY[:], w1b[:], y[:, h0 : h0 + 512])
                    n = min(512, L2 - h0)
                    nc.scalar.activation(r2[:, h0 : h0 + n], psY[:, 0:n],
                                         Relu, bias=b1bc)
                # conv2_2 data-stationary -> time-major h2, scattered j-major
                out_view = h2all[:, 128:].rearrange(
                    "p (j b c) -> p j b c", j=8, b=NB
                )[:, :, b, :]
                psH = wk_ps.tile([128, 128], F32, tag="ex", name="psH")
                for j in range(8):
                    mm(psH[:, 16 * j : 16 * j + 16],
                       r2[:, 128 * j : 128 * j + 128], w2b2[:])
                nc.scalar.activation(
                    out_view, psH[:].rearrange("p (j c) -> p j c", j=8), Copy)

            for it in range(NB + 3):
                if it >= 3:
                    emit_late(it - 3)
                if it < NB:
                    emit_convs(it)
                    emit_mdx(it)
                if 1 <= it <= NB:
                    emit_mid1(it - 1)
                    emit_exp_d(it - 1, 0)
                    emit_exp_m(it - 1, 1)
                    emit_exp_d(it - 1, 1)
                    emit_mid2(it - 1)

            # ---- signature2 (j-major time blocks, masked) ----
            psSH = uA_ps.tile([128, 1024], F32, tag="uA")
            mview = maskt[:].rearrange("p (j b) -> p j b", j=8).unsqueeze(3)
            d2v = d2[:].rearrange("p (j b c) -> p j b c", j=8, b=NB)
            d2fv = d2f[:].rearrange("p (j b c) -> p j b c", j=8, b=NB)
            for c in (0, 512):
                nc.tensor.matmul(psSH[:, c : c + 512], ssh[:],
                                 h2all[:, 128 + c : 640 + c], start=True,
                                 stop=False)
                nc.tensor.matmul(psSH[:, c : c + 512], s127[:],
                                 h2all[:, c : c + 512], start=False, stop=True)
                jsl = slice(c // 128, c // 128 + 4)
                nc.vector.tensor_tensor(
                    d2f[:, c : c + 512], h2all[:, 128 + c : 640 + c],
                    psSH[:, c : c + 512], SUB)
                nc.vector.tensor_tensor(
                    m2p[:, c : c + 512], h2all[:, 128 + c : 640 + c],
                    psSH[:, c : c + 512], ADD)
                nc.vector.tensor_tensor(
                    d2v[:, jsl], d2fv[:, jsl],
                    mview[:, jsl].broadcast_to((128, 4, NB, H2)), MUL)

            psB = wk_ps.tile([128, 128], F32, tag="ex", name="psB")
            for j in range(8):
                nc.tensor.matmul(psB[:], m2p[:, 128 * j : 128 * (j + 1)],
                                 d2[:, 128 * j : 128 * (j + 1)],
                                 start=j == 0, stop=j == 7)
            nc.vector.tensor_copy(f2sb[:], psB[:])

            for h0 in (0, 512):
                psF1 = wk_ps.tile([1, 512], F32, tag="ex", name="psF1")
                nc.tensor.matmul(psF1[:], onec[:], d2[:, h0 : h0 + 512],
                                 start=True, stop=True)
                nc.vector.tensor_copy(f1sb[:, h0 : h0 + 512], psF1[:])

            nc.sync.dma_start(out=f2o_d.ap(), in_=f2sb[:])
            nc.sync.dma_start(out=f1o_d.ap(), in_=f1sb[:])

    return nc


# ================= host-side preparation =================

def round_f32r(a):
    """round-to-nearest-even to 11-bit mantissa (fp32r)"""
    u = np.ascontiguousarray(a, np.float32).view(np.uint32)
    u = (u + 0x7FF + ((u >> 12) & 1)) & np.uint32(0xFFFFF000)
    return u.view(np.float32)


def _chunk_rows():
    """g-row layout: chunks of 64; chunk 7 = g-rows 448..483 at 0..35,
    dx rows 0..21 at 36..57, pad 58..63."""
    rows = []
    for c in range(NCH):
        loc = []
        for m in range(64):
            G = 64 * c + m
            if G < 484:
                loc.append(("g", G // C1, G % C1))
            elif G < 484 + C1:
                loc.append(("dx", None, G - 484))
            else:
                loc.append((None, None, None))
        rows.append(loc)
    return rows


def _prep_host(x, lengths):
    import ml_dtypes
    x = np.ascontiguousarray(x, dtype=np.float32)
    lengths = np.asarray(lengths).astype(np.int64)

    xim = np.zeros((B, 20, T), np.float32)
    for k in range(K):
        xim[:, 5 * k : 5 * k + 5, 0:L1] = x[:, k : k + L1, :].transpose(
            0, 2, 1)
    xs = np.zeros((B, 52, 512), np.float32)
    xs[:, 0:20, :] = xim[:, :, 0:512]
    xs[:, 32:52, :] = xim[:, :, 512:1024]
    xs = xs.astype(ml_dtypes.bfloat16)
    xh = np.ascontiguousarray(xim[:, 15:20, :])

    adj = (lengths - 2 * K + 2).astype(np.int64)
    tgrid = (np.arange(8)[None, :] * 128 + np.arange(128)[:, None])
    masks = []
    for core in range(NCORES):
        mcols = np.zeros((128, NB * 8), np.float32)
        for b in range(NB):
            a = min(int(adj[core * NB + b]), L2)
            for j in range(8):
                mcols[:, 8 * j + b] = (tgrid[:, j] < a).astype(np.float32)
        masks.append(mcols)
    return xs, xh, masks


def _prep_weights(inp):
    import ml_dtypes

    w = {}
    a1_w0 = inp["a1_w0"]
    w0p = np.zeros((20, H1), np.float32)
    for k in range(K):
        w0p[5 * k : 5 * k + 5, :] = a1_w0[:, :, k].T
    w["w0p"] = w0p
    cw0 = np.zeros((52, 64), np.float32)
    cw0[0:20] = w0p
    cw0[32:52] = w0p
    w["cw0"] = cw0
    w1a = inp["a1_w1"][:, :, 0].T.astype(np.float32)
    w["w1a"] = w1a
    w["cw1"] = np.concatenate([w1a, w1a], axis=0)
    w2a = inp["a1_w2"][:, :, 0].T.astype(np.float32)
    w["w2a"] = w2a
    w["cw2"] = np.concatenate([w2a, w2a], axis=0)
    w["w1b"] = inp["a2_w1"][:, :, 0].T.astype(np.float32)
    w["w2b2"] = inp["a2_w2"][:, :, 0].T.astype(np.float32)

    # suffix-summed conv weights over the a2 (484) block, device order
    w20 = inp["a2_w0"]  # (64, 506, 4)
    pidx = (np.array(PERM)[:, None] * C1 + np.array(PERM)[None, :]).reshape(-1)
    Wk_a = [w20[:, C1:, k].astype(np.float64)[:, pidx] for k in range(K)]
    V = [None] * 4
    V[3] = Wk_a[3]
    V[2] = Wk_a[2] + V[3]
    V[1] = Wk_a[1] + V[2]
    V[0] = Wk_a[0] + V[1]
    Wh = [w20[:, PERM, k].T.astype(np.float64) for k in range(K)]  # (22, 64)
    Vh = [None] * 4
    Vh[3] = Wh[3]
    Vh[2] = Wh[2] + Vh[3]
    Vh[1] = Wh[1] + Vh[2]
    Vh[0] = Wh[0] + Vh[1]

    rows = _chunk_rows()
    # expansion selectors: RI/RJ per pass [128, 128]
    for p in range(2):
        RI = np.zeros((128, 128), np.float32)
        RJ = np.zeros((128, 128), np.float32)
        for rb in range(4):
            for cg in range(2):
                c = 4 * p + 2 * (rb // 2) + cg
                for m in range(64):
                    kind, i, j = rows[c][m]
                    if kind == "g":
                        RI[32 * rb + i, 64 * cg + m] = 1.0
                        RJ[32 * rb + j, 64 * cg + m] = 1.0
                    elif kind == "dx":
                        RI[32 * rb + 22, 64 * cg + m] = 1.0  # ones row
                        RJ[32 * rb + j, 64 * cg + m] = 1.0
        w[f"ri{p}"] = RI
        w[f"rj{p}"] = RJ

    # u weights: VA_q = [v0|v1], VD_q = [v2|v3] over chunk pair (2q, 2q+1)
    for q in range(4):
        VA = np.zeros((128, 128), np.float64)
        VD = np.zeros((128, 128), np.float64)
        for half in range(2):
            c = 2 * q + half
            for m in range(64):
                kind, i, j = rows[c][m]
                G = 64 * c + m
                if kind == "g":
                    VA[64 * half + m, 0:64] = 0.5 * V[0][:, G]
                    VA[64 * half + m, 64:128] = 0.5 * V[1][:, G]
                    VD[64 * half + m, 0:64] = 0.5 * V[2][:, G]
                    VD[64 * half + m, 64:128] = 0.5 * V[3][:, G]
                elif kind == "dx":
                    VA[64 * half + m, 0:64] = Vh[0][j]
                    VA[64 * half + m, 64:128] = Vh[1][j]
                    VD[64 * half + m, 0:64] = Vh[2][j]
                    VD[64 * half + m, 64:128] = Vh[3][j]
        w[f"va{q}"] = VA
        w[f"vd{q}"] = VD

    ssh = np.zeros((128, 128), np.float32)
    for pp in range(1, 128):
        ssh[pp - 1, pp] = 1.0
    s127 = np.zeros((128, 128), np.float32)
    s127[127, 0] = 1.0
    w["ssh"] = ssh
    w["s127"] = s127
    w["onec"] = np.ones((128, 1), np.float32)

    trow = np.linspace(0.0, 1.0, L1, dtype=np.float32)
    trowA = trow[0:512].reshape(1, 512)
    trowB = np.zeros((1, 512), np.float32)
    trowB[0, 0 : L1 - 512] = trow[512:L1]
    b2 = np.asarray(inp["a2_b2"], np.float32)
    b2neg = np.zeros((1, 128), np.float32)
    b2neg[0] = -np.tile(b2, 8)
    w["trowA"], w["trowB"] = trowA, trowB
    w["b2negr"] = b2neg
    w["halfrow"] = np.full((1, 512), 0.5, np.float32)
    w["b0c"] = inp["a1_b0"].reshape(64, 1).astype(np.float32)
    w["b1c"] = inp["a1_b1"].reshape(64, 1).astype(np.float32)
    w["b1bc"] = inp["a2_b1"].reshape(64, 1).astype(np.float32)
    w["ba0c"] = inp["a2_b0"].reshape(64, 1).astype(np.float32)
    b2cc = np.zeros((48, 1), np.float32)
    b2cc[0:16, 0] = inp["a1_b2"]
    b2cc[32:48, 0] = inp["a1_b2"]
    w["b2c"] = b2cc
    w["b0r2"] = np.tile(inp["a1_b0"], 2).reshape(128, 1).astype(np.float32)
    w["b1r2"] = np.tile(inp["a1_b1"], 2).reshape(128, 1).astype(np.float32)
    zc = np.zeros((23, 1), np.float32)
    zc[22, 0] = 0.5
    w["zc23"] = zc

    cbf = np.zeros((128, CBF_COLS), ml_dtypes.bfloat16)
    for name, (r0, r, o, c) in CBF_LAYOUT.items():
        cbf[r0 : r0 + r, o : o + c] = np.asarray(w[name], np.float32).astype(
            ml_dtypes.bfloat16)
    cfr = np.zeros((128, CFR_COLS), np.float32)
    for name, (r0, r, o, c) in CFR_LAYOUT.items():
        cfr[r0 : r0 + r, o : o + c] = round_f32r(
            np.asarray(w[name], np.float32))
    cf32 = np.zeros((128, CF32_COLS), np.float32)
    for name, (r0, r, o, c) in CF32_LAYOUT.items():
        cf32[r0 : r0 + r, o : o + c] = np.asarray(
            w[name], np.float32).reshape(r, c)
    return {"cbf": cbf, "cfr": cfr, "cf32": cf32}


def kernel(**inputs):
    global _COMPILED
    x = np.asarray(inputs["x"], np.float32)
    lengths = np.asarray(inputs["lengths"])

    xs, xh, masks = _prep_host(x, lengths)
    w = _prep_weights({k: np.asarray(v) for k, v in inputs.items()})

    if _COMPILED is None:
        _c = build_program()
        _c.finalize()
        _COMPILED = _c
    nc = _COMPILED

    in_maps = []
    for core in range(NCORES):
        m = {"xs": xs[core * NB : (core + 1) * NB],
             "xh": xh[core * NB : (core + 1) * NB],
             "masktm": masks[core]}
        m.update(w)
        in_maps.append(m)

    _res = run_bass_kernel_spmd(nc, in_maps, list(range(NCORES)), trace=TRACE)
    globals()["LAST"] = _res
    res = _res.results

    # host: assemble s2 and final linear (+ bias correction on F2)
    lin_w = np.asarray(inputs["lin_w"], np.float32)
    lin_b = np.asarray(inputs["lin_b"], np.float32)
    b2 = np.asarray(inputs["a2_b2"], np.float32)
    out = np.zeros((B, OUT), np.float32)
    for core in range(NCORES):
        f2 = res[core]["f2o"]          # (128, 128)
        f1 = res[core]["f1o"][0]       # (T,) cols = 128j + 16b + ch
        f1r = f1.reshape(8, NB, H2)
        for b in range(NB):
            gb = core * NB + b
            F1 = f1r[:, b, :].sum(axis=0)
            F2 = (0.5 * f2[H2 * b : H2 * (b + 1), H2 * b : H2 * (b + 1)]
                  + np.outer(b2, F1))
            s2 = np.concatenate([F1, F2.reshape(-1)])
            out[gb] = s2 @ lin_w.T + lin_b
    return out.astype(np.float32)
